# revision 59
# baseline (speedup 1.0000x reference)
"""CFConv (SchNet continuous-filter convolution) on 8 TRN2 NeuronCores.

Reference computation:
    f    = x @ W_in                       # (20000, 128)
    f_j  = f[idx_j]                       # (640000, 128) gather
    wf   = w_ij * f_j                     # elementwise
    conv = segment_sum(wf, seg_i)         # (20000, 128), seg_i sorted
    out  = conv @ W_out + b_out

Distribution: seg_i is sorted, so atoms are split into 8 contiguous
ranges of 2560 (padded to 20480); each core gets the edges targeting its
atom range.  No collectives needed — each core owns its output rows.

Per-core device pipeline (matmuls bf16 x bf16 or bf16 x fp8, f32 PSUM):
  Phase A: f = x @ W_in computed locally (replicated), written to two
           internal HBM half-tables (bf16 rows, split so phase B can
           start after only the low half is ready).
  Phase B: edges processed in groups of 128 (one group = one matmul
           contraction).  Groups are host-packed per 128-atom window,
           split into lo/hi f-table halves, each padded to fixed group
           counts k_lo/k_hi so the graph is identical on all cores (the
           dma_gather descriptor generation on the Q7 cores is the
           kernel's throughput floor at ~2.2ns/idx).
    - w_ij group tiles DMA'd from HBM (host-reordered, bf16)
    - f_j rows fetched with gpsimd.dma_gather (MoE gather primitive),
      two calls per window striped across all 4 SWDGE queues so the
      four Q7 core pairs generate descriptors in parallel
    - wf = w * f_j on VectorE
    - segment-sum via TensorE: psum[fm, atom_window] += wf_g^T @ S_g
      where S_g is the host-built 0/1 edge->atom one-hot (fp8 rhs)
    - out^T = W_out^T @ conv^T (TensorE), bias via ScalarE, transposed
      back per 128x128 tile on TensorE, DMA'd to the output shard.

Atoms are host-relabeled (snake-deal by per-atom edge count) so every
window carries a near-equal edge count, minimizing the uniform padding;
the output is un-permuted on the host after the gather.

Measured on 8 axon TRN2 cores: ~309-330 us HW exec on quiet runs
(shared-HBM neighbor noise can add 10-20%), rel err 5.2e-3 vs the f32
reference.
"""

import numpy as np
import ml_dtypes

import concourse.bacc as bacc
import concourse.bass as bass
import concourse.mybir as mybir
import concourse.tile as tile
from concourse.bass_utils import run_bass_kernel_spmd

BF16 = ml_dtypes.bfloat16
FP8 = ml_dtypes.float8_e4m3

N_ATOMS = 20000
N_EDGES = 640000
F = 128
N_CORES = 8
A_CORE = 2560                 # padded atoms per core
A_PAD = A_CORE * N_CORES      # 20480
CHUNK = 512                   # atoms per PSUM chunk (one bank)
WIN = 128                     # atoms per window (matmul N dim)
WIN_PER_CORE = A_CORE // WIN  # 20
N_WIN = A_PAD // WIN          # 160

TRACE = False                 # set True (with ntff shim) for profiling
_BUILD_CACHE: dict = {}


def _build(k_lo: int, k_hi: int):
    """Build the SPMD Bass graph for given per-window group counts.

    Each 128-atom window's edges are split into a lo half (f rows
    [0, A_PAD/2)) and a hi half, each padded to k_lo/k_hi groups of 128;
    the two dma_gathers per window depend only on their half of the f
    table, so phase B overlaps the tail of phase A.
    """
    key = (k_lo, k_hi)
    if key in _BUILD_CACHE:
        return _BUILD_CACHE[key]

    k_fix = k_lo + k_hi
    G = WIN_PER_CORE * k_fix      # groups per core
    E = G * 128                   # padded edges per core
    H = A_PAD // 2
    bf = mybir.dt.bfloat16
    f32 = mybir.dt.float32

    nc = bacc.Bacc("TRN2", target_bir_lowering=False, debug=False,
                   num_swdge_queues=4, num_devices=N_CORES)
    xT_e = nc.dram_tensor("xT", [128, A_PAD], bf, kind="ExternalInput")
    w_in_e = nc.dram_tensor("w_in", [128, 128], bf, kind="ExternalInput")
    w_out_e = nc.dram_tensor("w_out", [128, 128], bf, kind="ExternalInput")
    b_e = nc.dram_tensor("b_out", [128, 1], f32, kind="ExternalInput")
    id_e = nc.dram_tensor("ident", [128, 128], bf, kind="ExternalInput")
    w_ed_e = nc.dram_tensor("w_ed", [128, G, F], bf, kind="ExternalInput")
    s_ed_e = nc.dram_tensor("s_ed", [128, G, WIN], mybir.dt.float8e4,
                            kind="ExternalInput")
    idx_e = nc.dram_tensor("idxw", [128, E // 16], mybir.dt.int16,
                           kind="ExternalInput")
    out_e = nc.dram_tensor("out", [A_CORE, F], f32, kind="ExternalOutput")

    with tile.TileContext(nc) as tc:
        with (
            tc.tile_pool(name="dram", bufs=1, space="DRAM") as dpool,
            tc.tile_pool(name="const", bufs=1) as cpool,
            tc.tile_pool(name="pha", bufs=3) as apool,
            tc.tile_pool(name="psA", bufs=3, space="PSUM") as psA,
        ):
            f_lo_hbm = dpool.tile([H, F], bf)
            f_hi_hbm = dpool.tile([H, F], bf)

            w_in_t = cpool.tile([128, 128], bf)
            nc.sync.dma_start(w_in_t[:], w_in_e[:])
            w_out_t = cpool.tile([128, 128], bf)
            nc.sync.dma_start(w_out_t[:], w_out_e[:])
            b_t = cpool.tile([128, 1], f32)
            nc.sync.dma_start(b_t[:], b_e[:])
            id_t = cpool.tile([128, 128], bf)
            nc.sync.dma_start(id_t[:], id_e[:])
            idx_t = cpool.tile([128, E // 16], mybir.dt.int16)
            nc.scalar.dma_start(idx_t[:], idx_e[:])

            # ---------------- Phase A: f table ----------------
            # All four xq loads are issued up front so none of them queues
            # behind compute-gated f-stores on the Sync HWDGE FIFO.
            QW = A_PAD // 4
            f_sb = None
            xq = []
            for x4 in range(4):
                xq_t = apool.tile([128, QW], bf, tag="xq")
                nc.sync.dma_start(xq_t[:], xT_e[:, x4 * QW:(x4 + 1) * QW])
                xq.append(xq_t)
            for x4 in range(4):
                xq_t = xq[x4]
                for t4q in range(QW // 512):
                    t4 = x4 * (QW // 512) + t4q
                    ps = psA.tile([128, 4, 128], f32)
                    for q in range(4):
                        tl = t4q * 4 + q
                        nc.tensor.matmul(
                            ps[:, q, :],
                            xq_t[:, tl * 128:(tl + 1) * 128],
                            w_in_t[:],
                            start=True, stop=True,
                        )
                    j = t4 % 2
                    if j == 0:
                        f_sb = apool.tile([128, 8, F], bf, tag="fsb")
                    nc.vector.tensor_copy(
                        f_sb[:, j * 4:(j + 1) * 4, :], ps[:])
                    if j == 1:
                        a0 = (t4 - 1) * 512
                        tgt = f_lo_hbm if a0 < H else f_hi_hbm
                        a0 = a0 % H
                        dst = tgt[a0:a0 + 1024, :].rearrange(
                            "(j p) f -> p j f", p=128)
                        nc.sync.dma_start(dst, f_sb[:])

            # ---------------- Phase B: edges ----------------
            with (
                tc.tile_pool(name="phb", bufs=3) as bpool,
                tc.tile_pool(name="fjp", bufs=6) as fjpool,
                tc.tile_pool(name="psC", bufs=2, space="PSUM") as pscp,
                tc.tile_pool(name="ps2", bufs=2, space="PSUM") as ps2p,
                tc.tile_pool(name="ps3", bufs=1, space="PSUM") as ps3p,
            ):
                psc = None
                for wk in range(WIN_PER_CORE):
                    ch = wk // 4
                    col = WIN * (wk % 4)

                    w_t = bpool.tile([128, k_fix, F], bf, tag="w")
                    nc.scalar.dma_start(
                        w_t[:], w_ed_e[:, wk * k_fix:(wk + 1) * k_fix, :])
                    s_t = bpool.tile([128, k_fix, WIN], mybir.dt.float8e4,
                                     tag="s")
                    nc.scalar.dma_start(
                        s_t[:], s_ed_e[:, wk * k_fix:(wk + 1) * k_fix, :])
                    base8 = wk * k_fix * 8
                    fj_t = fjpool.tile([128, k_fix, F], bf, tag="fj")
                    nc.gpsimd.dma_gather(
                        fj_t[:, 0:k_lo, :], f_lo_hbm[:, :],
                        idx_t[:, base8:base8 + k_lo * 8],
                        num_idxs=k_lo * 128,
                        num_idxs_reg=k_lo * 128,
                        elem_size=F,
                        single_packet=False,
                        queue_num=(2 * wk) % 4,
                    )
                    nc.gpsimd.dma_gather(
                        fj_t[:, k_lo:k_fix, :], f_hi_hbm[:, :],
                        idx_t[:, base8 + k_lo * 8:base8 + k_fix * 8],
                        num_idxs=k_hi * 128,
                        num_idxs_reg=k_hi * 128,
                        elem_size=F,
                        single_packet=False,
                        queue_num=(2 * wk + 1) % 4,
                    )
                    wf_t = bpool.tile([128, k_fix, F], bf, tag="wf")
                    nc.vector.tensor_tensor(
                        wf_t[:], w_t[:], fj_t[:], mybir.AluOpType.mult)

                    if wk % 4 == 0:
                        psc = pscp.tile([128, CHUNK], f32)
                    for g in range(k_fix):
                        nc.tensor.matmul(
                            psc[:, col:col + WIN],
                            wf_t[:, g, :],
                            s_t[:, g, :],
                            start=(g == 0), stop=(g == k_fix - 1),
                        )

                    if wk % 4 == 3:
                        convT = bpool.tile([128, CHUNK], bf, tag="convT")
                        nc.vector.tensor_copy(convT[:], psc[:])
                        ps2 = ps2p.tile([128, CHUNK], f32)
                        nc.tensor.matmul(ps2[:], w_out_t[:], convT[:],
                                         start=True, stop=True)
                        outT = bpool.tile([128, CHUNK], bf, tag="outT")
                        nc.scalar.activation(
                            outT[:], ps2[:],
                            mybir.ActivationFunctionType.Identity,
                            bias=b_t[:],
                        )
                        outf = bpool.tile([128, 4, F], f32, tag="outf")
                        for t in range(4):
                            ps3 = ps3p.tile([128, 128], bf)
                            nc.tensor.transpose(
                                ps3[:], outT[:, t * 128:(t + 1) * 128],
                                id_t[:])
                            nc.vector.tensor_copy(outf[:, t, :], ps3[:])
                        dst = out_e[ch * CHUNK:(ch + 1) * CHUNK, :].rearrange(
                            "(t p) f -> p t f", p=128)
                        nc.sync.dma_start(dst, outf[:])

    nc.compile()
    _BUILD_CACHE[key] = nc
    return nc


def _prep(x, w_ij, seg_i, idx_j, W_in, W_out, b_out):
    """Host-side sharding: reorder/pad edges, build S one-hots, wrap idxs."""
    x = np.asarray(x, dtype=np.float32)
    w_ij = np.asarray(w_ij, dtype=np.float32)
    seg = np.asarray(seg_i).astype(np.int64)
    idxj = np.asarray(idx_j).astype(np.int64)

    # Relabel atoms so every 128-atom window gets a near-equal edge count
    # (snake-deal atoms in decreasing edge-count order over the windows).
    # This minimizes the uniform per-window padding k_lo/k_hi, which sets
    # the dma_gather descriptor-generation floor.  seg/idx/x/f-table/output
    # all permute consistently; the output is un-permuted on the host.
    cnt = np.bincount(seg, minlength=N_ATOMS)
    order = np.argsort(-cnt, kind="stable")
    i = np.arange(N_ATOMS)
    r, c = np.divmod(i, N_WIN)
    w = np.where(r % 2 == 0, c, N_WIN - 1 - c)
    perm = np.empty(N_ATOMS, np.int64)
    perm[order] = w * WIN + r
    seg = perm[seg]
    idxj = perm[idxj]
    o = np.argsort(seg, kind="stable")
    seg, idxj, w_ij = seg[o], idxj[o], w_ij[o]

    bounds = np.searchsorted(seg, np.arange(N_WIN + 1) * WIN)
    Hs = A_PAD // 2

    # per-window lo/hi split (f-table halves)
    lo_ids, hi_ids, lo_v, hi_v = [], [], [], []
    n_lo = np.zeros(N_WIN, np.int64)
    n_hi = np.zeros(N_WIN, np.int64)
    for k in range(N_WIN):
        b0, b1 = bounds[k], bounds[k + 1]
        ids = np.arange(b0, b1)
        v = idxj[b0:b1]
        m = v < Hs
        lo_ids.append(ids[m])
        hi_ids.append(ids[~m])
        lo_v.append(v[m].astype(np.int16))
        hi_v.append((v[~m] - Hs).astype(np.int16))
        n_lo[k] = m.sum()
        n_hi[k] = (~m).sum()
    k_lo = max(1, int(np.ceil(n_lo.max() / 128)))
    k_hi = max(1, int(np.ceil(n_hi.max() / 128)))
    k_fix = k_lo + k_hi
    e_win = k_fix * 128
    g_core = WIN_PER_CORE * k_fix
    e_pad = g_core * 128

    # padded edge-id + gather-idx matrices in lo|hi order
    eidx = np.zeros((N_WIN, e_win), np.int64)
    valid = np.zeros((N_WIN, e_win), bool)
    gidx = np.zeros((N_WIN, e_win), np.int16)
    for k in range(N_WIN):
        a, b = n_lo[k], n_hi[k]
        eidx[k, :a] = lo_ids[k]
        valid[k, :a] = True
        gidx[k, :a] = lo_v[k]
        off = k_lo * 128
        eidx[k, off:off + b] = hi_ids[k]
        valid[k, off:off + b] = True
        gidx[k, off:off + b] = hi_v[k]

    w_bf = w_ij.astype(BF16)

    xT = np.zeros((128, A_PAD), BF16)
    xT[:, perm] = np.ascontiguousarray(x.T).astype(BF16)
    shared = {
        "xT": xT,
        "w_in": np.asarray(W_in, np.float32).astype(BF16),
        "w_out": np.asarray(W_out, np.float32).astype(BF16),
        "b_out": np.asarray(b_out, np.float32).reshape(128, 1).copy(),
        "ident": np.eye(128, dtype=BF16),
    }

    in_maps = []
    for c in range(N_CORES):
        sl = slice(c * WIN_PER_CORE, (c + 1) * WIN_PER_CORE)
        ei = eidx[sl].reshape(-1)
        va = valid[sl].reshape(-1)

        w_rows = np.zeros((e_pad, F), BF16)
        w_rows[va] = w_bf[ei[va]]
        w_ed = np.ascontiguousarray(
            w_rows.reshape(g_core, 128, F).transpose(1, 0, 2))

        wb = (np.arange(c * WIN_PER_CORE, (c + 1) * WIN_PER_CORE)
              * WIN).repeat(e_win)
        rel = seg[ei] - wb
        s_rows = np.zeros((e_pad, WIN), FP8)
        vrows = np.nonzero(va)[0]
        s_rows[vrows, rel[vrows]] = 1
        s_ed = np.ascontiguousarray(
            s_rows.reshape(g_core, 128, WIN).transpose(1, 0, 2))

        # wrapped idx layout, one wrap per gather call (lo and hi per window)
        gi = gidx[sl]                              # [20, e_win]
        blocks = []
        for wkk in range(WIN_PER_CORE):
            blocks.append(gi[wkk, :k_lo * 128].reshape(-1, 16).T)
            blocks.append(gi[wkk, k_lo * 128:].reshape(-1, 16).T)
        idxw = np.ascontiguousarray(
            np.tile(np.concatenate(blocks, axis=1), (8, 1)))

        m = dict(shared)
        m["w_ed"] = w_ed
        m["s_ed"] = s_ed
        m["idxw"] = idxw
        in_maps.append(m)
    return k_lo, k_hi, in_maps, perm


def kernel(x, w_ij, seg_i, idx_j, seg_i_sum, W_in, W_out, b_out):
    k_lo, k_hi, in_maps, perm = _prep(x, w_ij, seg_i, idx_j, W_in, W_out,
                                      b_out)
    nc = _build(k_lo, k_hi)
    res = run_bass_kernel_spmd(nc, in_maps, core_ids=list(range(N_CORES)),
                               trace=TRACE)
    kernel.last_result = res
    out = np.concatenate(
        [np.asarray(res.results[c]["out"]) for c in range(N_CORES)], axis=0)
    return np.ascontiguousarray(out[perm]).astype(np.float32)


# revision 60
# speedup vs baseline: 1.1514x; 1.1514x over previous
"""CFConv (SchNet continuous-filter convolution) on 8 TRN2 NeuronCores.

Reference computation:
    f    = x @ W_in                       # (20000, 128)
    f_j  = f[idx_j]                       # (640000, 128) gather
    wf   = w_ij * f_j                     # elementwise
    conv = segment_sum(wf, seg_i)         # (20000, 128), seg_i sorted
    out  = conv @ W_out + b_out

Distribution: seg_i is sorted, so atoms are split into 8 contiguous
ranges of 2560 (padded to 20480); each core gets the edges targeting its
atom range.  No collectives needed — each core owns its output rows.

Per-core device pipeline (matmuls bf16 x bf16 or bf16 x fp8, f32 PSUM):
  Phase A: f = x @ W_in computed locally (replicated), written to two
           internal HBM half-tables (bf16 rows, split so phase B can
           start after only the low half is ready).
  Phase B: edges processed in groups of 128 (one group = one matmul
           contraction).  Groups are host-packed per 128-atom window,
           split into lo/hi f-table halves, each padded to fixed group
           counts k_lo/k_hi so the graph is identical on all cores (the
           dma_gather descriptor generation on the Q7 cores is the
           kernel's throughput floor at ~2.2ns/idx).
    - w_ij group tiles DMA'd from HBM (host-reordered, bf16)
    - f_j rows fetched with gpsimd.dma_gather (MoE gather primitive),
      two calls per window striped across all 4 SWDGE queues so the
      four Q7 core pairs generate descriptors in parallel
    - wf = w * f_j on VectorE
    - segment-sum via TensorE: psum[fm, atom_window] += wf_g^T @ S_g
      where S_g is the host-built 0/1 edge->atom one-hot (fp8 rhs)
    - out^T = W_out^T @ conv^T (TensorE), bias via ScalarE, transposed
      back per 128x128 tile on TensorE, DMA'd to the output shard.

Atoms are host-relabeled (snake-deal by per-atom edge count) so every
window carries a near-equal edge count, minimizing the uniform padding;
the output is un-permuted on the host after the gather.

Measured on 8 axon TRN2 cores: ~309-330 us HW exec on quiet runs
(shared-HBM neighbor noise can add 10-20%), rel err 5.2e-3 vs the f32
reference.
"""

import numpy as np
import ml_dtypes

import concourse.bacc as bacc
import concourse.bass as bass
import concourse.mybir as mybir
import concourse.tile as tile
from concourse.bass_utils import run_bass_kernel_spmd

BF16 = ml_dtypes.bfloat16
FP8 = ml_dtypes.float8_e4m3

N_ATOMS = 20000
N_EDGES = 640000
F = 128
N_CORES = 8
A_CORE = 2560                 # padded atoms per core
A_PAD = A_CORE * N_CORES      # 20480
CHUNK = 512                   # atoms per PSUM chunk (one bank)
WIN = 128                     # atoms per window (matmul N dim)
WIN_PER_CORE = A_CORE // WIN  # 20
N_WIN = A_PAD // WIN          # 160

TRACE = False                 # set True (with ntff shim) for profiling
_BUILD_CACHE: dict = {}


def _build(k_lo: int, k_hi: int):
    """Build the SPMD Bass graph for given per-window group counts.

    Each 128-atom window's edges are split into a lo half (f rows
    [0, A_PAD/2)) and a hi half, each padded to k_lo/k_hi groups of 128;
    the two dma_gathers per window depend only on their half of the f
    table, so phase B overlaps the tail of phase A.
    """
    key = (k_lo, k_hi)
    if key in _BUILD_CACHE:
        return _BUILD_CACHE[key]

    k_fix = k_lo + k_hi
    G = WIN_PER_CORE * k_fix      # groups per core
    E = G * 128                   # padded edges per core
    H = A_PAD // 2
    bf = mybir.dt.bfloat16
    f32 = mybir.dt.float32

    nc = bacc.Bacc("TRN2", target_bir_lowering=False, debug=False,
                   num_swdge_queues=4, num_devices=N_CORES)
    xT_e = nc.dram_tensor("xT", [128, A_PAD], bf, kind="ExternalInput")
    w_in_e = nc.dram_tensor("w_in", [128, 128], bf, kind="ExternalInput")
    w_out_e = nc.dram_tensor("w_out", [128, 128], bf, kind="ExternalInput")
    b_e = nc.dram_tensor("b_out", [128, 1], f32, kind="ExternalInput")
    id_e = nc.dram_tensor("ident", [128, 128], bf, kind="ExternalInput")
    w_ed_e = nc.dram_tensor("w_ed", [128, G, F], bf, kind="ExternalInput")
    s_ed_e = nc.dram_tensor("s_ed", [128, G, WIN], mybir.dt.float8e4,
                            kind="ExternalInput")
    idx_e = nc.dram_tensor("idxw", [128, E // 16], mybir.dt.int16,
                           kind="ExternalInput")
    out_e = nc.dram_tensor("out", [A_CORE, F], f32, kind="ExternalOutput")

    with tile.TileContext(nc) as tc:
        with (
            tc.tile_pool(name="dram", bufs=1, space="DRAM") as dpool,
            tc.tile_pool(name="const", bufs=1) as cpool,
            tc.tile_pool(name="pha", bufs=3) as apool,
            tc.tile_pool(name="psA", bufs=3, space="PSUM") as psA,
        ):
            f_lo_hbm = dpool.tile([H, F], bf)
            f_hi_hbm = dpool.tile([H, F], bf)

            w_in_t = cpool.tile([128, 128], bf)
            nc.sync.dma_start(w_in_t[:], w_in_e[:])
            w_out_t = cpool.tile([128, 128], bf)
            nc.sync.dma_start(w_out_t[:], w_out_e[:])
            b_t = cpool.tile([128, 1], f32)
            nc.sync.dma_start(b_t[:], b_e[:])
            id_t = cpool.tile([128, 128], bf)
            nc.sync.dma_start(id_t[:], id_e[:])
            idx_t = cpool.tile([128, E // 16], mybir.dt.int16)
            nc.scalar.dma_start(idx_t[:], idx_e[:])

            # ---------------- Phase A: f table ----------------
            # All four xq loads are issued up front so none of them queues
            # behind compute-gated f-stores on the Sync HWDGE FIFO.
            QW = A_PAD // 4
            f_sb = None
            xq = []
            for x4 in range(3):
                xq_t = apool.tile([128, QW], bf, tag="xq")
                nc.sync.dma_start(xq_t[:], xT_e[:, x4 * QW:(x4 + 1) * QW])
                xq.append(xq_t)
            xq3_t = None
            for x4 in range(4):
                xq_t = xq[x4] if x4 < 3 else xq3_t
                for t4q in range(QW // 512):
                    t4 = x4 * (QW // 512) + t4q
                    ps = psA.tile([128, 4, 128], f32)
                    for q in range(4):
                        tl = t4q * 4 + q
                        nc.tensor.matmul(
                            ps[:, q, :],
                            xq_t[:, tl * 128:(tl + 1) * 128],
                            w_in_t[:],
                            start=True, stop=True,
                        )
                    j = t4 % 2
                    if j == 0:
                        f_sb = apool.tile([128, 8, F], bf, tag="fsb")
                    nc.vector.tensor_copy(
                        f_sb[:, j * 4:(j + 1) * 4, :], ps[:])
                    if j == 1:
                        a0 = (t4 - 1) * 512
                        tgt = f_lo_hbm if a0 < H else f_hi_hbm
                        a0 = a0 % H
                        dst = tgt[a0:a0 + 1024, :].rearrange(
                            "(j p) f -> p j f", p=128)
                        nc.sync.dma_start(dst, f_sb[:])
                if x4 == 0:
                    xq3_t = apool.tile([128, QW], bf, tag="xq")
                    nc.sync.dma_start(xq3_t[:], xT_e[:, 3 * QW:4 * QW])

            # ---------------- Phase B: edges ----------------
            with (
                tc.tile_pool(name="phb", bufs=3) as bpool,
                tc.tile_pool(name="fjp", bufs=6) as fjpool,
                tc.tile_pool(name="psC", bufs=2, space="PSUM") as pscp,
                tc.tile_pool(name="ps2", bufs=2, space="PSUM") as ps2p,
                tc.tile_pool(name="ps3", bufs=1, space="PSUM") as ps3p,
            ):
                psc = None
                for wk in range(WIN_PER_CORE):
                    ch = wk // 4
                    col = WIN * (wk % 4)

                    w_t = bpool.tile([128, k_fix, F], bf, tag="w")
                    nc.scalar.dma_start(
                        w_t[:], w_ed_e[:, wk * k_fix:(wk + 1) * k_fix, :])
                    s_t = bpool.tile([128, k_fix, WIN], mybir.dt.float8e4,
                                     tag="s")
                    nc.scalar.dma_start(
                        s_t[:], s_ed_e[:, wk * k_fix:(wk + 1) * k_fix, :])
                    base8 = wk * k_fix * 8
                    fj_t = fjpool.tile([128, k_fix, F], bf, tag="fj")
                    nc.gpsimd.dma_gather(
                        fj_t[:, 0:k_lo, :], f_lo_hbm[:, :],
                        idx_t[:, base8:base8 + k_lo * 8],
                        num_idxs=k_lo * 128,
                        num_idxs_reg=k_lo * 128,
                        elem_size=F,
                        single_packet=False,
                        queue_num=(2 * wk) % 4,
                    )
                    nc.gpsimd.dma_gather(
                        fj_t[:, k_lo:k_fix, :], f_hi_hbm[:, :],
                        idx_t[:, base8 + k_lo * 8:base8 + k_fix * 8],
                        num_idxs=k_hi * 128,
                        num_idxs_reg=k_hi * 128,
                        elem_size=F,
                        single_packet=False,
                        queue_num=(2 * wk + 1) % 4,
                    )
                    wf_t = bpool.tile([128, k_fix, F], bf, tag="wf")
                    nc.vector.tensor_tensor(
                        wf_t[:], w_t[:], fj_t[:], mybir.AluOpType.mult)

                    if wk % 4 == 0:
                        psc = pscp.tile([128, CHUNK], f32)
                    for g in range(k_fix):
                        nc.tensor.matmul(
                            psc[:, col:col + WIN],
                            wf_t[:, g, :],
                            s_t[:, g, :],
                            start=(g == 0), stop=(g == k_fix - 1),
                        )

                    if wk % 4 == 3:
                        convT = bpool.tile([128, CHUNK], bf, tag="convT")
                        nc.vector.tensor_copy(convT[:], psc[:])
                        ps2 = ps2p.tile([128, CHUNK], f32)
                        nc.tensor.matmul(ps2[:], w_out_t[:], convT[:],
                                         start=True, stop=True)
                        outT = bpool.tile([128, CHUNK], bf, tag="outT")
                        nc.scalar.activation(
                            outT[:], ps2[:],
                            mybir.ActivationFunctionType.Identity,
                            bias=b_t[:],
                        )
                        outf = bpool.tile([128, 4, F], f32, tag="outf")
                        for t in range(4):
                            ps3 = ps3p.tile([128, 128], bf)
                            nc.tensor.transpose(
                                ps3[:], outT[:, t * 128:(t + 1) * 128],
                                id_t[:])
                            nc.vector.tensor_copy(outf[:, t, :], ps3[:])
                        dst = out_e[ch * CHUNK:(ch + 1) * CHUNK, :].rearrange(
                            "(t p) f -> p t f", p=128)
                        nc.sync.dma_start(dst, outf[:])

    nc.compile()
    _BUILD_CACHE[key] = nc
    return nc


def _prep(x, w_ij, seg_i, idx_j, W_in, W_out, b_out):
    """Host-side sharding: reorder/pad edges, build S one-hots, wrap idxs."""
    x = np.asarray(x, dtype=np.float32)
    w_ij = np.asarray(w_ij, dtype=np.float32)
    seg = np.asarray(seg_i).astype(np.int64)
    idxj = np.asarray(idx_j).astype(np.int64)

    # Relabel atoms so every 128-atom window gets a near-equal edge count
    # (snake-deal atoms in decreasing edge-count order over the windows).
    # This minimizes the uniform per-window padding k_lo/k_hi, which sets
    # the dma_gather descriptor-generation floor.  seg/idx/x/f-table/output
    # all permute consistently; the output is un-permuted on the host.
    cnt = np.bincount(seg, minlength=N_ATOMS)
    order = np.argsort(-cnt, kind="stable")
    i = np.arange(N_ATOMS)
    r, c = np.divmod(i, N_WIN)
    w = np.where(r % 2 == 0, c, N_WIN - 1 - c)
    perm = np.empty(N_ATOMS, np.int64)
    perm[order] = w * WIN + r
    seg = perm[seg]
    idxj = perm[idxj]
    o = np.argsort(seg, kind="stable")
    seg, idxj, w_ij = seg[o], idxj[o], w_ij[o]

    bounds = np.searchsorted(seg, np.arange(N_WIN + 1) * WIN)
    Hs = A_PAD // 2

    # per-window lo/hi split (f-table halves)
    lo_ids, hi_ids, lo_v, hi_v = [], [], [], []
    n_lo = np.zeros(N_WIN, np.int64)
    n_hi = np.zeros(N_WIN, np.int64)
    for k in range(N_WIN):
        b0, b1 = bounds[k], bounds[k + 1]
        ids = np.arange(b0, b1)
        v = idxj[b0:b1]
        m = v < Hs
        lo_ids.append(ids[m])
        hi_ids.append(ids[~m])
        lo_v.append(v[m].astype(np.int16))
        hi_v.append((v[~m] - Hs).astype(np.int16))
        n_lo[k] = m.sum()
        n_hi[k] = (~m).sum()
    k_lo = max(1, int(np.ceil(n_lo.max() / 128)))
    k_hi = max(1, int(np.ceil(n_hi.max() / 128)))
    k_fix = k_lo + k_hi
    e_win = k_fix * 128
    g_core = WIN_PER_CORE * k_fix
    e_pad = g_core * 128

    # padded edge-id + gather-idx matrices in lo|hi order
    eidx = np.zeros((N_WIN, e_win), np.int64)
    valid = np.zeros((N_WIN, e_win), bool)
    gidx = np.zeros((N_WIN, e_win), np.int16)
    for k in range(N_WIN):
        a, b = n_lo[k], n_hi[k]
        eidx[k, :a] = lo_ids[k]
        valid[k, :a] = True
        gidx[k, :a] = lo_v[k]
        off = k_lo * 128
        eidx[k, off:off + b] = hi_ids[k]
        valid[k, off:off + b] = True
        gidx[k, off:off + b] = hi_v[k]

    w_bf = w_ij.astype(BF16)

    xT = np.zeros((128, A_PAD), BF16)
    xT[:, perm] = np.ascontiguousarray(x.T).astype(BF16)
    shared = {
        "xT": xT,
        "w_in": np.asarray(W_in, np.float32).astype(BF16),
        "w_out": np.asarray(W_out, np.float32).astype(BF16),
        "b_out": np.asarray(b_out, np.float32).reshape(128, 1).copy(),
        "ident": np.eye(128, dtype=BF16),
    }

    in_maps = []
    for c in range(N_CORES):
        sl = slice(c * WIN_PER_CORE, (c + 1) * WIN_PER_CORE)
        ei = eidx[sl].reshape(-1)
        va = valid[sl].reshape(-1)

        w_rows = np.zeros((e_pad, F), BF16)
        w_rows[va] = w_bf[ei[va]]
        w_ed = np.ascontiguousarray(
            w_rows.reshape(g_core, 128, F).transpose(1, 0, 2))

        wb = (np.arange(c * WIN_PER_CORE, (c + 1) * WIN_PER_CORE)
              * WIN).repeat(e_win)
        rel = seg[ei] - wb
        s_rows = np.zeros((e_pad, WIN), FP8)
        vrows = np.nonzero(va)[0]
        s_rows[vrows, rel[vrows]] = 1
        s_ed = np.ascontiguousarray(
            s_rows.reshape(g_core, 128, WIN).transpose(1, 0, 2))

        # wrapped idx layout, one wrap per gather call (lo and hi per window)
        gi = gidx[sl]                              # [20, e_win]
        blocks = []
        for wkk in range(WIN_PER_CORE):
            blocks.append(gi[wkk, :k_lo * 128].reshape(-1, 16).T)
            blocks.append(gi[wkk, k_lo * 128:].reshape(-1, 16).T)
        idxw = np.ascontiguousarray(
            np.tile(np.concatenate(blocks, axis=1), (8, 1)))

        m = dict(shared)
        m["w_ed"] = w_ed
        m["s_ed"] = s_ed
        m["idxw"] = idxw
        in_maps.append(m)
    return k_lo, k_hi, in_maps, perm


def kernel(x, w_ij, seg_i, idx_j, seg_i_sum, W_in, W_out, b_out):
    k_lo, k_hi, in_maps, perm = _prep(x, w_ij, seg_i, idx_j, W_in, W_out,
                                      b_out)
    nc = _build(k_lo, k_hi)
    res = run_bass_kernel_spmd(nc, in_maps, core_ids=list(range(N_CORES)),
                               trace=TRACE)
    kernel.last_result = res
    out = np.concatenate(
        [np.asarray(res.results[c]["out"]) for c in range(N_CORES)], axis=0)
    return np.ascontiguousarray(out[perm]).astype(np.float32)


# revision 61
# speedup vs baseline: 1.1862x; 1.0303x over previous
"""CFConv (SchNet continuous-filter convolution) on 8 TRN2 NeuronCores.

Reference computation:
    f    = x @ W_in                       # (20000, 128)
    f_j  = f[idx_j]                       # (640000, 128) gather
    wf   = w_ij * f_j                     # elementwise
    conv = segment_sum(wf, seg_i)         # (20000, 128), seg_i sorted
    out  = conv @ W_out + b_out

Distribution: seg_i is sorted, so atoms are split into 8 contiguous
ranges of 2560 (padded to 20480); each core gets the edges targeting its
atom range.  No collectives needed — each core owns its output rows.

Per-core device pipeline (matmuls bf16 x bf16 or bf16 x fp8, f32 PSUM):
  Phase A: f = x @ W_in computed locally (replicated), written to two
           internal HBM half-tables (bf16 rows, split so phase B can
           start after only the low half is ready).
  Phase B: edges processed in groups of 128 (one group = one matmul
           contraction).  Groups are host-packed per 128-atom window,
           split into lo/hi f-table halves, each padded to fixed group
           counts k_lo/k_hi so the graph is identical on all cores (the
           dma_gather descriptor generation on the Q7 cores is the
           kernel's throughput floor at ~2.2ns/idx).
    - w_ij group tiles DMA'd from HBM (host-reordered, bf16)
    - f_j rows fetched with gpsimd.dma_gather (MoE gather primitive),
      two calls per window striped across all 4 SWDGE queues so the
      four Q7 core pairs generate descriptors in parallel
    - wf = w * f_j on VectorE
    - segment-sum via TensorE: psum[fm, atom_window] += wf_g^T @ S_g
      where S_g is the host-built 0/1 edge->atom one-hot (fp8 rhs)
    - out^T = W_out^T @ conv^T (TensorE), bias via ScalarE, transposed
      back per 128x128 tile on TensorE, DMA'd to the output shard.

Atoms are host-relabeled (snake-deal by per-atom edge count) so every
window carries a near-equal edge count, minimizing the uniform padding;
the output is un-permuted on the host after the gather.

Measured on 8 axon TRN2 cores: ~309-330 us HW exec on quiet runs
(shared-HBM neighbor noise can add 10-20%), rel err 5.2e-3 vs the f32
reference.
"""

import numpy as np
import ml_dtypes

import concourse.bacc as bacc
import concourse.bass as bass
import concourse.mybir as mybir
import concourse.tile as tile
from concourse.bass_utils import run_bass_kernel_spmd

BF16 = ml_dtypes.bfloat16
FP8 = ml_dtypes.float8_e4m3

N_ATOMS = 20000
N_EDGES = 640000
F = 128
N_CORES = 8
A_CORE = 2560                 # padded atoms per core
A_PAD = A_CORE * N_CORES      # 20480
CHUNK = 512                   # atoms per PSUM chunk (one bank)
WIN = 128                     # atoms per window (matmul N dim)
WIN_PER_CORE = A_CORE // WIN  # 20
N_WIN = A_PAD // WIN          # 160

TRACE = False                 # set True (with ntff shim) for profiling
_BUILD_CACHE: dict = {}


def _build(k_lo: int, k_hi: int):
    """Build the SPMD Bass graph for given per-window group counts.

    Each 128-atom window's edges are split into a lo half (f rows
    [0, A_PAD/2)) and a hi half, each padded to k_lo/k_hi groups of 128;
    the two dma_gathers per window depend only on their half of the f
    table, so phase B overlaps the tail of phase A.
    """
    key = (k_lo, k_hi)
    if key in _BUILD_CACHE:
        return _BUILD_CACHE[key]

    k_fix = k_lo + k_hi
    G = WIN_PER_CORE * k_fix      # groups per core
    E = G * 128                   # padded edges per core
    H = A_PAD // 2
    bf = mybir.dt.bfloat16
    f32 = mybir.dt.float32

    nc = bacc.Bacc("TRN2", target_bir_lowering=False, debug=False,
                   num_swdge_queues=4, num_devices=N_CORES)
    xT_e = nc.dram_tensor("xT", [128, A_PAD], bf, kind="ExternalInput")
    w_in_e = nc.dram_tensor("w_in", [128, 128], bf, kind="ExternalInput")
    w_out_e = nc.dram_tensor("w_out", [128, 128], bf, kind="ExternalInput")
    b_e = nc.dram_tensor("b_out", [128, 1], f32, kind="ExternalInput")
    id_e = nc.dram_tensor("ident", [128, 128], bf, kind="ExternalInput")
    w_ed_e = nc.dram_tensor("w_ed", [128, G, F], bf, kind="ExternalInput")
    s_ed_e = nc.dram_tensor("s_ed", [128, G, WIN], mybir.dt.float8e4,
                            kind="ExternalInput")
    idx_e = nc.dram_tensor("idxw", [128, E // 16], mybir.dt.int16,
                           kind="ExternalInput")
    out_e = nc.dram_tensor("out", [A_CORE, F], f32, kind="ExternalOutput")

    with tile.TileContext(nc) as tc:
        with (
            tc.tile_pool(name="dram", bufs=1, space="DRAM") as dpool,
            tc.tile_pool(name="const", bufs=1) as cpool,
            tc.tile_pool(name="pha", bufs=3) as apool,
            tc.tile_pool(name="psA", bufs=3, space="PSUM") as psA,
        ):
            f_lo_hbm = dpool.tile([H, F], bf)
            f_hi_hbm = dpool.tile([H, F], bf)

            w_in_t = cpool.tile([128, 128], bf)
            nc.sync.dma_start(w_in_t[:], w_in_e[:])
            w_out_t = cpool.tile([128, 128], bf)
            nc.sync.dma_start(w_out_t[:], w_out_e[:])
            b_t = cpool.tile([128, 1], f32)
            nc.sync.dma_start(b_t[:], b_e[:])
            id_t = cpool.tile([128, 128], bf)
            nc.sync.dma_start(id_t[:], id_e[:])
            idx_t = cpool.tile([128, E // 16], mybir.dt.int16)
            nc.scalar.dma_start(idx_t[:], idx_e[:])

            # ---------------- Phase A: f table ----------------
            # All four xq loads are issued up front so none of them queues
            # behind compute-gated f-stores on the Sync HWDGE FIFO.
            QW = A_PAD // 4
            f_sb = None
            for x4 in range(4):
                xq_t = apool.tile([128, QW], bf, tag="xq")
                nc.sync.dma_start(xq_t[:], xT_e[:, x4 * QW:(x4 + 1) * QW])
                for t4q in range(QW // 512):
                    t4 = x4 * (QW // 512) + t4q
                    ps = psA.tile([128, 4, 128], f32)
                    for q in range(4):
                        tl = t4q * 4 + q
                        nc.tensor.matmul(
                            ps[:, q, :],
                            xq_t[:, tl * 128:(tl + 1) * 128],
                            w_in_t[:],
                            start=True, stop=True,
                        )
                    j = t4 % 2
                    if j == 0:
                        f_sb = apool.tile([128, 8, F], bf, tag="fsb")
                    nc.vector.tensor_copy(
                        f_sb[:, j * 4:(j + 1) * 4, :], ps[:])
                    if j == 1:
                        a0 = (t4 - 1) * 512
                        tgt = f_lo_hbm if a0 < H else f_hi_hbm
                        a0 = a0 % H
                        dst = tgt[a0:a0 + 1024, :].rearrange(
                            "(j p) f -> p j f", p=128)
                        nc.sync.dma_start(dst, f_sb[:])

            # ---------------- Phase B: edges ----------------
            with (
                tc.tile_pool(name="phb", bufs=3) as bpool,
                tc.tile_pool(name="fjp", bufs=6) as fjpool,
                tc.tile_pool(name="psC", bufs=2, space="PSUM") as pscp,
                tc.tile_pool(name="ps2", bufs=2, space="PSUM") as ps2p,
                tc.tile_pool(name="ps3", bufs=1, space="PSUM") as ps3p,
            ):
                psc = None
                for wk in range(WIN_PER_CORE):
                    ch = wk // 4
                    col = WIN * (wk % 4)

                    w_t = bpool.tile([128, k_fix, F], bf, tag="w")
                    nc.scalar.dma_start(
                        w_t[:], w_ed_e[:, wk * k_fix:(wk + 1) * k_fix, :])
                    s_t = bpool.tile([128, k_fix, WIN], mybir.dt.float8e4,
                                     tag="s")
                    nc.scalar.dma_start(
                        s_t[:], s_ed_e[:, wk * k_fix:(wk + 1) * k_fix, :])
                    base8 = wk * k_fix * 8
                    fj_t = fjpool.tile([128, k_fix, F], bf, tag="fj")
                    nc.gpsimd.dma_gather(
                        fj_t[:, 0:k_lo, :], f_lo_hbm[:, :],
                        idx_t[:, base8:base8 + k_lo * 8],
                        num_idxs=k_lo * 128,
                        num_idxs_reg=k_lo * 128,
                        elem_size=F,
                        single_packet=False,
                        queue_num=(2 * wk) % 4,
                    )
                    nc.gpsimd.dma_gather(
                        fj_t[:, k_lo:k_fix, :], f_hi_hbm[:, :],
                        idx_t[:, base8 + k_lo * 8:base8 + k_fix * 8],
                        num_idxs=k_hi * 128,
                        num_idxs_reg=k_hi * 128,
                        elem_size=F,
                        single_packet=False,
                        queue_num=(2 * wk + 1) % 4,
                    )
                    wf_t = bpool.tile([128, k_fix, F], bf, tag="wf")
                    nc.vector.tensor_tensor(
                        wf_t[:], w_t[:], fj_t[:], mybir.AluOpType.mult)

                    if wk % 4 == 0:
                        psc = pscp.tile([128, CHUNK], f32)
                    for g in range(k_fix):
                        nc.tensor.matmul(
                            psc[:, col:col + WIN],
                            wf_t[:, g, :],
                            s_t[:, g, :],
                            start=(g == 0), stop=(g == k_fix - 1),
                        )

                    if wk % 4 == 3:
                        convT = bpool.tile([128, CHUNK], bf, tag="convT")
                        nc.vector.tensor_copy(convT[:], psc[:])
                        ps2 = ps2p.tile([128, CHUNK], f32)
                        nc.tensor.matmul(ps2[:], w_out_t[:], convT[:],
                                         start=True, stop=True)
                        outT = bpool.tile([128, CHUNK], bf, tag="outT")
                        nc.scalar.activation(
                            outT[:], ps2[:],
                            mybir.ActivationFunctionType.Identity,
                            bias=b_t[:],
                        )
                        outf = bpool.tile([128, 4, F], f32, tag="outf")
                        for t in range(4):
                            ps3 = ps3p.tile([128, 128], bf)
                            nc.tensor.transpose(
                                ps3[:], outT[:, t * 128:(t + 1) * 128],
                                id_t[:])
                            nc.vector.tensor_copy(outf[:, t, :], ps3[:])
                        dst = out_e[ch * CHUNK:(ch + 1) * CHUNK, :].rearrange(
                            "(t p) f -> p t f", p=128)
                        nc.sync.dma_start(dst, outf[:])

    nc.compile()
    _BUILD_CACHE[key] = nc
    return nc


def _prep(x, w_ij, seg_i, idx_j, W_in, W_out, b_out):
    """Host-side sharding: reorder/pad edges, build S one-hots, wrap idxs."""
    x = np.asarray(x, dtype=np.float32)
    w_ij = np.asarray(w_ij, dtype=np.float32)
    seg = np.asarray(seg_i).astype(np.int64)
    idxj = np.asarray(idx_j).astype(np.int64)

    # Relabel atoms so every 128-atom window gets a near-equal edge count
    # (snake-deal atoms in decreasing edge-count order over the windows).
    # This minimizes the uniform per-window padding k_lo/k_hi, which sets
    # the dma_gather descriptor-generation floor.  seg/idx/x/f-table/output
    # all permute consistently; the output is un-permuted on the host.
    cnt = np.bincount(seg, minlength=N_ATOMS)
    order = np.argsort(-cnt, kind="stable")
    i = np.arange(N_ATOMS)
    r, c = np.divmod(i, N_WIN)
    w = np.where(r % 2 == 0, c, N_WIN - 1 - c)
    perm = np.empty(N_ATOMS, np.int64)
    perm[order] = w * WIN + r
    seg = perm[seg]
    idxj = perm[idxj]
    o = np.argsort(seg, kind="stable")
    seg, idxj, w_ij = seg[o], idxj[o], w_ij[o]

    bounds = np.searchsorted(seg, np.arange(N_WIN + 1) * WIN)
    Hs = A_PAD // 2

    # per-window lo/hi split (f-table halves)
    lo_ids, hi_ids, lo_v, hi_v = [], [], [], []
    n_lo = np.zeros(N_WIN, np.int64)
    n_hi = np.zeros(N_WIN, np.int64)
    for k in range(N_WIN):
        b0, b1 = bounds[k], bounds[k + 1]
        ids = np.arange(b0, b1)
        v = idxj[b0:b1]
        m = v < Hs
        lo_ids.append(ids[m])
        hi_ids.append(ids[~m])
        lo_v.append(v[m].astype(np.int16))
        hi_v.append((v[~m] - Hs).astype(np.int16))
        n_lo[k] = m.sum()
        n_hi[k] = (~m).sum()
    k_lo = max(1, int(np.ceil(n_lo.max() / 128)))
    k_hi = max(1, int(np.ceil(n_hi.max() / 128)))
    k_fix = k_lo + k_hi
    e_win = k_fix * 128
    g_core = WIN_PER_CORE * k_fix
    e_pad = g_core * 128

    # padded edge-id + gather-idx matrices in lo|hi order
    eidx = np.zeros((N_WIN, e_win), np.int64)
    valid = np.zeros((N_WIN, e_win), bool)
    gidx = np.zeros((N_WIN, e_win), np.int16)
    for k in range(N_WIN):
        a, b = n_lo[k], n_hi[k]
        eidx[k, :a] = lo_ids[k]
        valid[k, :a] = True
        gidx[k, :a] = lo_v[k]
        off = k_lo * 128
        eidx[k, off:off + b] = hi_ids[k]
        valid[k, off:off + b] = True
        gidx[k, off:off + b] = hi_v[k]

    w_bf = w_ij.astype(BF16)

    xT = np.zeros((128, A_PAD), BF16)
    xT[:, perm] = np.ascontiguousarray(x.T).astype(BF16)
    shared = {
        "xT": xT,
        "w_in": np.asarray(W_in, np.float32).astype(BF16),
        "w_out": np.asarray(W_out, np.float32).astype(BF16),
        "b_out": np.asarray(b_out, np.float32).reshape(128, 1).copy(),
        "ident": np.eye(128, dtype=BF16),
    }

    in_maps = []
    for c in range(N_CORES):
        sl = slice(c * WIN_PER_CORE, (c + 1) * WIN_PER_CORE)
        ei = eidx[sl].reshape(-1)
        va = valid[sl].reshape(-1)

        w_rows = np.zeros((e_pad, F), BF16)
        w_rows[va] = w_bf[ei[va]]
        w_ed = np.ascontiguousarray(
            w_rows.reshape(g_core, 128, F).transpose(1, 0, 2))

        wb = (np.arange(c * WIN_PER_CORE, (c + 1) * WIN_PER_CORE)
              * WIN).repeat(e_win)
        rel = seg[ei] - wb
        s_rows = np.zeros((e_pad, WIN), FP8)
        vrows = np.nonzero(va)[0]
        s_rows[vrows, rel[vrows]] = 1
        s_ed = np.ascontiguousarray(
            s_rows.reshape(g_core, 128, WIN).transpose(1, 0, 2))

        # wrapped idx layout, one wrap per gather call (lo and hi per window)
        gi = gidx[sl]                              # [20, e_win]
        blocks = []
        for wkk in range(WIN_PER_CORE):
            blocks.append(gi[wkk, :k_lo * 128].reshape(-1, 16).T)
            blocks.append(gi[wkk, k_lo * 128:].reshape(-1, 16).T)
        idxw = np.ascontiguousarray(
            np.tile(np.concatenate(blocks, axis=1), (8, 1)))

        m = dict(shared)
        m["w_ed"] = w_ed
        m["s_ed"] = s_ed
        m["idxw"] = idxw
        in_maps.append(m)
    return k_lo, k_hi, in_maps, perm


def kernel(x, w_ij, seg_i, idx_j, seg_i_sum, W_in, W_out, b_out):
    k_lo, k_hi, in_maps, perm = _prep(x, w_ij, seg_i, idx_j, W_in, W_out,
                                      b_out)
    nc = _build(k_lo, k_hi)
    res = run_bass_kernel_spmd(nc, in_maps, core_ids=list(range(N_CORES)),
                               trace=TRACE)
    kernel.last_result = res
    out = np.concatenate(
        [np.asarray(res.results[c]["out"]) for c in range(N_CORES)], axis=0)
    return np.ascontiguousarray(out[perm]).astype(np.float32)
